# revision 1
# baseline (speedup 1.0000x reference)
"""GMM log-likelihood kernel for Trainium2 (Bass/Tile), 8-core data-parallel.

Math (host precompute in f64):
  B_k = L_k^{-1} (Cholesky inverse),  w_k = B_k^T B_k mu_k
  wlp_k(x) = -0.5*||B_k x||^2 + w_k . x + C_k
  lse(x)   = m0 + log(sum_k exp(wlp_k - m0))   (m0 = global shift, safe:
             measured per-sample max wlp spread is ~37 nats << f32 exp range)
  out      = sum_x lse(x)

Per core: the [25000, 64] data slice (zero-padded to 196 tiles of 128
samples) is processed in pairs of tiles: PE transposes each pair into a
[128,128] stationary (two 64-row feature blocks), then row-packed bf16
matmuls against the replicated moving operand [B_all | W] produce
Y [128 samples, 1024] + lin [128, 16] per tile.  ACT squares Y out of
PSUM, DVE group-reduces the squares to per-component norms and assembles
wlp into a [128, 196*16] buffer.  A batched phase 2 does exp /
component-sum / log / masked accumulate, and a ones-matmul folds the 128
partitions into the final scalar.  Host sums the 8 per-core scalars.
"""

import numpy as np

N_COMPONENTS = 16
N_FEATURES = 64
N_SAMPLES = 200000
N_CORES = 8
PER_CORE = N_SAMPLES // N_CORES          # 25000
TILE_P = 128
N_TILES = -(-PER_CORE // TILE_P)         # 196 (ceil)
N_PAIRS = (N_TILES + 1) // 2             # 98
PADDED = N_TILES * TILE_P                # 25088
KD = N_COMPONENTS * N_FEATURES           # 1024

_CACHE = {}


def _build_nc(n_pairs):
    import concourse.tile as tile
    from concourse import bacc, mybir

    n_tiles = n_pairs * 2
    padded = n_tiles * TILE_P
    f32 = mybir.dt.float32
    bf16 = mybir.dt.bfloat16

    nc = bacc.Bacc("TRN2", target_bir_lowering=False, debug=False,
                   num_devices=N_CORES)

    xp = nc.dram_tensor("xp", [padded, N_FEATURES], bf16, kind="ExternalInput").ap()
    bmov2 = nc.dram_tensor("bmov2", [128, KD + N_COMPONENTS], bf16,
                           kind="ExternalInput").ap()
    cq = nc.dram_tensor("cq", [1, N_COMPONENTS], f32, kind="ExternalInput").ap()
    oner = nc.dram_tensor("oner", [1, 128], f32, kind="ExternalInput").ap()
    mask = nc.dram_tensor("mask", [128, n_tiles], f32, kind="ExternalInput").ap()
    ident = nc.dram_tensor("ident", [128, 128], bf16, kind="ExternalInput").ap()
    ones = nc.dram_tensor("ones", [128, 1], f32, kind="ExternalInput").ap()
    out = nc.dram_tensor("out", [1, 1], f32, kind="ExternalOutput").ap()

    W = n_tiles * N_COMPONENTS

    with tile.TileContext(nc) as tc:
        with (
            tc.tile_pool(name="const", bufs=1) as const_pool,
            tc.tile_pool(name="wbuf", bufs=1) as wbuf_pool,
            tc.tile_pool(name="xin", bufs=4) as xin_pool,
            tc.tile_pool(name="xt", bufs=3) as xt_pool,
            tc.tile_pool(name="ysq", bufs=2) as ysq_pool,
            tc.tile_pool(name="sm", bufs=4) as sm_pool,
            tc.tile_pool(name="tp", bufs=2, space="PSUM") as tp_pool,
            tc.tile_pool(name="yp", bufs=2, space="PSUM") as yp_pool,
            tc.tile_pool(name="lp", bufs=2, space="PSUM") as lp_pool,
        ):
            bm = const_pool.tile([128, KD + N_COMPONENTS], bf16)
            nc.sync.dma_start(bm[:], bmov2[:])
            cqs = const_pool.tile([1, N_COMPONENTS], f32)
            nc.sync.dma_start(cqs[:], cq[:])
            onr = const_pool.tile([1, 128], f32)
            nc.sync.dma_start(onr[:], oner[:])
            msks = const_pool.tile([128, n_tiles], f32)
            nc.sync.dma_start(msks[:], mask[:])
            idn = const_pool.tile([128, 128], bf16)
            nc.sync.dma_start(idn[:], ident[:])
            on1 = const_pool.tile([128, 1], f32)
            nc.sync.dma_start(on1[:], ones[:])

            wbuf = wbuf_pool.tile([128, W], f32)
            ebuf = wbuf_pool.tile([128, W], f32)

            for p in range(n_pairs):
                xpair = xin_pool.tile([128, 128], bf16, tag="xpair")
                r0 = (2 * p) * TILE_P
                nc.sync.dma_start(xpair[:, 0:64], xp[r0:r0 + 128, :])
                nc.sync.dma_start(xpair[:, 64:128], xp[r0 + 128:r0 + 256, :])

                tp = tp_pool.tile([128, 128], bf16, tag="tp")
                nc.tensor.transpose(tp[:], xpair[:], idn[:])
                xt = xt_pool.tile([128, 128], bf16, tag="xt")
                nc.scalar.copy(xt[:], tp[:])

                ysq = ysq_pool.tile([128, 2 * KD], f32, tag="ysq")
                lps = []
                for h in range(2):
                    hp = h * 64
                    yp = yp_pool.tile([128, KD], f32, tag="yp")
                    lp = lp_pool.tile([128, N_COMPONENTS], f32, tag="lp")
                    lhs = xt[hp:hp + 64, :]
                    nc.tensor.matmul(yp[:, 0:512], lhs, bm[hp:hp + 64, 0:512])
                    nc.tensor.matmul(yp[:, 512:1024], lhs, bm[hp:hp + 64, 512:1024])
                    nc.tensor.matmul(lp[:], lhs, bm[hp:hp + 64, 1024:1040],
                                     start=True, stop=False)
                    nc.tensor.matmul(lp[:], onr[:], cqs[:],
                                     start=False, stop=True)
                    nc.scalar.activation(ysq[:, h * KD:(h + 1) * KD], yp[:],
                                         mybir.ActivationFunctionType.Square)
                    lps.append(lp)

                st = sm_pool.tile([128, 2 * N_COMPONENTS], f32, tag="st")
                nc.vector.reduce_sum(
                    st[:],
                    ysq[:].rearrange("p (k i) -> p k i", i=N_FEATURES),
                    axis=mybir.AxisListType.X)

                for h in range(2):
                    col = (2 * p + h) * N_COMPONENTS
                    nc.vector.scalar_tensor_tensor(
                        wbuf[:, col:col + N_COMPONENTS],
                        st[:, h * N_COMPONENTS:(h + 1) * N_COMPONENTS],
                        -0.5, lps[h][:],
                        op0=mybir.AluOpType.mult, op1=mybir.AluOpType.add)

            # phase 2
            nc.scalar.activation(ebuf[:], wbuf[:],
                                 mybir.ActivationFunctionType.Exp)
            rsum = const_pool.tile([128, n_tiles], f32)
            nc.vector.reduce_sum(
                rsum[:],
                ebuf[:].rearrange("p (t k) -> p t k", k=N_COMPONENTS),
                axis=mybir.AxisListType.X)
            lnr = const_pool.tile([128, n_tiles], f32)
            nc.scalar.activation(lnr[:], rsum[:],
                                 mybir.ActivationFunctionType.Ln)
            msum = const_pool.tile([128, n_tiles], f32)
            nc.vector.tensor_mul(msum[:], lnr[:], msks[:])
            csum = const_pool.tile([128, 1], f32)
            nc.vector.reduce_sum(csum[:], msum[:], axis=mybir.AxisListType.X)

            rp = tp_pool.tile([1, 1], f32, tag="tp")
            nc.tensor.matmul(rp[:], on1[:], csum[:])
            res = const_pool.tile([1, 1], f32)
            nc.scalar.copy(res[:], rp[:])
            nc.sync.dma_start(out[:], res[:])

    nc.compile()
    return nc


def _precompute(weights, means, covariances):
    """Host-side O(K d^3) prep in float64. Returns (bmov2, cq_row, m0)."""
    import ml_dtypes

    K, d = means.shape
    L = np.linalg.cholesky(covariances.astype(np.float64))
    half_logdet = np.log(np.diagonal(L, axis1=-2, axis2=-1)).sum(-1)
    eye = np.eye(d)
    B = np.stack([np.linalg.solve(L[k], eye) for k in range(K)])  # L^-1
    mu = means.astype(np.float64)
    c = np.einsum('kij,kj->ki', B, mu)
    w_lin = np.einsum('kij,ki->kj', B, c)
    r = (c * c).sum(-1)
    const = (np.log(weights.astype(np.float64))
             - 0.5 * d * np.log(2.0 * np.pi) - half_logdet)
    C = const - 0.5 * r
    m0 = float(C.max()) - 20.0

    bmov = np.zeros((d, K * d + K), np.float32)
    for k in range(K):
        bmov[:, k * d:(k + 1) * d] = B[k].T.astype(np.float32)
    bmov[:, K * d:] = w_lin.T.astype(np.float32)
    bmov2 = np.vstack([bmov, bmov]).astype(ml_dtypes.bfloat16)   # [128, 1040]
    cq_row = (C - m0).astype(np.float32)                         # [16]
    return bmov2, cq_row, m0


def _make_inputs(data, bmov2, cq_row, n_tiles):
    """Build the 8 per-core input maps for the padded per-core data slices."""
    import ml_dtypes

    padded = n_tiles * TILE_P
    cq = cq_row[None, :].astype(np.float32)
    oner = np.ones((1, 128), np.float32)
    mask = np.zeros((128, n_tiles), np.float32)
    for t in range(n_tiles):
        v = min(max(PER_CORE - t * TILE_P, 0), TILE_P)
        mask[:v, t] = 1.0
    ident = np.eye(128, dtype=ml_dtypes.bfloat16)
    ones = np.ones((128, 1), np.float32)

    in_maps = []
    for c in range(N_CORES):
        sl = data[c * PER_CORE:(c + 1) * PER_CORE]
        xp = np.zeros((padded, N_FEATURES), ml_dtypes.bfloat16)
        xp[:sl.shape[0]] = sl.astype(ml_dtypes.bfloat16)
        in_maps.append({"xp": xp, "bmov2": bmov2, "cq": cq, "mask": mask,
                        "ident": ident, "ones": ones, "oner": oner})
    return in_maps


def _run(data, weights, means, covariances, trace=False):
    from concourse.bass_utils import run_bass_kernel_spmd

    data = np.asarray(data, np.float32)
    bmov2, cq_row, m0 = _precompute(np.asarray(weights), np.asarray(means),
                                    np.asarray(covariances))
    if "nc" not in _CACHE:
        _CACHE["nc"] = _build_nc(N_PAIRS)
    nc = _CACHE["nc"]

    in_maps = _make_inputs(data, bmov2, cq_row, N_TILES)
    res = run_bass_kernel_spmd(nc, in_maps, list(range(N_CORES)), trace=trace)
    total = 0.0
    for c in range(N_CORES):
        total += float(res.results[c]["out"][0, 0]) + PER_CORE * m0
    return np.float32(total), res


def kernel(data, weights, means, covariances):
    return _run(data, weights, means, covariances)[0]



# revision 7
# speedup vs baseline: 1.5684x; 1.5684x over previous
"""GMM log-likelihood kernel for Trainium2 (Bass/Tile), 8-core data-parallel.

Math (host precompute in f64):
  B_k = L_k^{-1} / sqrt(2),  c_k = B_k mu_k
  S_k(x) = ||B_k x - c_k||^2 = 0.5 * maha_k(x)
  wlp_k(x) = C_k - m0 - S_k(x),  C_k = log w_k - d/2 log 2pi - half_logdet_k
  out = sum_x [ m0 + log sum_k exp(wlp_k(x)) ]

Per core (25000 samples, padded to 196 tiles of 128):
  Data is host-transposed to xp [66, 25088] bf16: rows 0:64 = x^T, rows
  64/65 = 1.0 (the matching bm rows carry -c_k hi/lo parts, so the PE
  matmul emits Y = B x - c directly -- no PE transpose, no lin matmul).
  Per tile: matmul lhsT = xp-slice [66,128], rhs = bm [66,1024] ->
  Y PSUM [128 samples, 1024 (k,j)], two tiles per PSUM buffer.
  Squares (0.5 y^2 via prescale) are split between ACT (Square -> bf16,
  whole pair per call) and a custom DVE op sq(a)+sq(b) that folds the
  64->32 reduction while reading PSUM.  A bf16 tensor_tensor add tree
  (2x_1p DVE mode) reduces 32->2 per (tile,k) batched over 14-tile
  groups; two STTs finish S and wlp = const - S (const read replicated).
  Phase 2: exp / k-group reduce / ln / mask / reduce / ones-matmul fold.
Host sums the 8 per-core scalars (+ m0 per real sample).
"""

import numpy as np

N_COMPONENTS = 16
N_FEATURES = 64
N_SAMPLES = 200000
N_CORES = 8
PER_CORE = N_SAMPLES // N_CORES          # 25000
TILE_P = 128
N_TILES = -(-PER_CORE // TILE_P)         # 196 (ceil)
PADDED = N_TILES * TILE_P                # 25088
KD = N_COMPONENTS * N_FEATURES           # 1024
CONTR = N_FEATURES + 2                   # 66 (features + ones-row hi/lo)

GROUP_TILES = 14                         # tiles per tree batch
N_GROUPS = N_TILES // GROUP_TILES        # 14
PAIRS_PER_GROUP = GROUP_TILES // 2       # 7
N_PAIRS = N_TILES // 2                   # 98
CHUNK_TILES = 28                         # DMA chunk = 2 groups
N_CHUNKS = N_TILES // CHUNK_TILES        # 7

# pair index -> path: "H" pairs split the square between ACT (first half
# of each k-group) and the DVE custom op (second half, fused +add);
# "A" pairs let ACT square everything.  (ACT/DVE busy-balance knob.)
PAIR_PERIOD = 9
H_PAIR_MOD = (0, 2, 4, 6)

_CACHE = {}


def _register_sq2():
    """Register the custom DVE op  out = sq(in0) + in1  (f32 internal).

    in0 reads a raw-Y half from PSUM (DVE may read only ONE PSUM operand),
    in1 reads the ACT-pre-squared other half from SBUF."""
    from concourse import dve_ops
    from concourse.dve_spec import Spec, Src0, Src1, sq, lower, _has_src1
    from concourse.dve_uop import DveOpSpec

    if any(op.name == "SQ1_ADD_ANT" for op in dve_ops.OPS):
        return next(op for op in dve_ops.OPS if op.name == "SQ1_ADD_ANT")
    spec = Spec(
        body=sq(Src0) + Src1,
        reference=lambda in0, in1, s0, s1, imm2: (
            in0.astype(np.float32) ** 2 + in1.astype(np.float32)),
    )
    shas = {}
    for ver in ("v3", "v4"):
        c = DveOpSpec(name="SQ1_ADD_ANT", opcode=17,
                      uops=lower(spec, ver=ver), rd1_en=_has_src1(spec))
        shas[ver] = c.sha(ver)
    op = dve_ops.DveOp("SQ1_ADD_ANT", spec, subdim=False, uops_sha=shas)
    row = max(dve_ops._SUB_OPCODE_FOR_NAME.values()) + 1
    assert row < 0x20
    dve_ops.OPS.append(op)
    dve_ops._SUB_OPCODE_FOR_NAME[op.name] = row
    dve_ops.CUSTOM_DVE_SPECS[op.name] = spec
    return op


def _build_nc():
    import concourse.tile as tile
    from concourse import bacc, mybir

    sq2 = _register_sq2()

    f32 = mybir.dt.float32
    bf16 = mybir.dt.bfloat16
    W = GROUP_TILES * N_COMPONENTS       # 224 wlp columns per group

    nc = bacc.Bacc("TRN2", target_bir_lowering=False, debug=False,
                   num_devices=N_CORES)

    xp = nc.dram_tensor("xp", [CONTR, PADDED], bf16, kind="ExternalInput").ap()
    bm = nc.dram_tensor("bm", [CONTR, KD], bf16, kind="ExternalInput").ap()
    lconst = nc.dram_tensor("lconst", [128, W], f32, kind="ExternalInput").ap()
    mask = nc.dram_tensor("mask", [128, N_TILES], f32, kind="ExternalInput").ap()
    ones = nc.dram_tensor("ones", [128, 1], f32, kind="ExternalInput").ap()
    out = nc.dram_tensor("out", [1, 1], f32, kind="ExternalOutput").ap()

    K = N_COMPONENTS

    with tile.TileContext(nc) as tc:
        with (
            tc.tile_pool(name="const", bufs=1) as const_pool,
            tc.tile_pool(name="xin", bufs=2) as xin_pool,
            tc.tile_pool(name="ysq", bufs=3) as ysq_pool,
            tc.tile_pool(name="sq32", bufs=2) as sq32_pool,
            tc.tile_pool(name="tree", bufs=2) as tree_pool,
            tc.tile_pool(name="wb", bufs=1) as wb_pool,
            tc.tile_pool(name="yp", bufs=2, space="PSUM") as yp_pool,
        ):
            bms = const_pool.tile([CONTR, KD], bf16)
            nc.sync.dma_start(bms[:], bm[:])
            lcs = const_pool.tile([128, W], f32)
            nc.sync.dma_start(lcs[:], lconst[:])
            msks = const_pool.tile([128, N_TILES], f32)
            nc.sync.dma_start(msks[:], mask[:])
            on1 = const_pool.tile([128, 1], f32)
            nc.sync.dma_start(on1[:], ones[:])

            wlp = wb_pool.tile([128, N_TILES * K], f32)
            ebuf = wb_pool.tile([128, N_TILES * K], f32)

            for c in range(N_CHUNKS):
                xb = xin_pool.tile([CONTR, CHUNK_TILES * TILE_P], bf16,
                                   tag="xb")
                c0 = c * CHUNK_TILES * TILE_P
                nc.sync.dma_start(xb[:], xp[:, c0:c0 + CHUNK_TILES * TILE_P])

                for gl in range(2):              # two 14-tile groups per chunk
                    g = 2 * c + gl
                    sq32 = sq32_pool.tile([128, GROUP_TILES * K * 32], bf16,
                                          tag="sq32")
                    for pl in range(PAIRS_PER_GROUP):
                        pair = g * PAIRS_PER_GROUP + pl
                        yp = yp_pool.tile([128, 2 * KD], f32, tag="yp")
                        for h in range(2):
                            tl = (gl * GROUP_TILES + 2 * pl + h) * TILE_P
                            lhs = xb[:, tl:tl + TILE_P]
                            nc.tensor.matmul(yp[:, h * KD:h * KD + 512],
                                             lhs, bms[:, 0:512])
                            nc.tensor.matmul(yp[:, h * KD + 512:h * KD + KD],
                                             lhs, bms[:, 512:KD])
                        sqc = (2 * pl) * K * 32
                        ypv = yp[:].rearrange("p (t k i) -> p (t k) i", i=64,
                                              k=K)
                        if pair % PAIR_PERIOD in H_PAIR_MOD:
                            # H pair: ACT squares the 0:32 halves -> bf16,
                            # DVE custom squares 32:64 from PSUM + adds
                            ysa = ysq_pool.tile([128, KD], bf16, tag="ysa")
                            nc.scalar.activation(
                                ysa[:].rearrange("p (t i) -> p t i", i=32),
                                ypv[:, :, 0:32],
                                mybir.ActivationFunctionType.Square)
                            for h in range(2):
                                nc.vector._custom_dve(
                                    sq2,
                                    out=sq32[:, sqc + h * K * 32:
                                             sqc + (h + 1) * K * 32]
                                    .rearrange("p (k i) -> p k i", i=32),
                                    in0=yp[:, h * KD:(h + 1) * KD]
                                    .rearrange("p (k i) -> p k i", i=64)
                                    [:, :, 32:64],
                                    in1=ysa[:, h * K * 32:(h + 1) * K * 32]
                                    .rearrange("p (k i) -> p k i", i=32),
                                )
                        else:
                            # A pair: ACT squares whole pair -> bf16, then one
                            # 2x-mode TT add folds 64->32
                            ysq = ysq_pool.tile([128, 2 * KD], bf16, tag="ysq")
                            nc.scalar.activation(
                                ysq[:], yp[:],
                                mybir.ActivationFunctionType.Square)
                            yv = ysq[:].rearrange("p (k i) -> p k i", i=64)
                            nc.vector.tensor_add(
                                sq32[:, sqc:sqc + 2 * K * 32]
                                .rearrange("p (k i) -> p k i", i=32),
                                yv[:, :, 0:32], yv[:, :, 32:64])

                    # tree: 32 -> 16 -> 8 -> 4 -> 2 (bf16 TT @2x), then STTs
                    t16 = tree_pool.tile([128, W * 16], bf16, tag="t16")
                    v = sq32[:].rearrange("p (w i) -> p w i", i=32)
                    nc.vector.tensor_add(
                        t16[:].rearrange("p (w i) -> p w i", i=16),
                        v[:, :, 0:16], v[:, :, 16:32])
                    t8 = tree_pool.tile([128, W * 8], bf16, tag="t8")
                    v = t16[:].rearrange("p (w i) -> p w i", i=16)
                    nc.vector.tensor_add(
                        t8[:].rearrange("p (w i) -> p w i", i=8),
                        v[:, :, 0:8], v[:, :, 8:16])
                    t4 = tree_pool.tile([128, W * 4], bf16, tag="t4")
                    v = t8[:].rearrange("p (w i) -> p w i", i=8)
                    nc.vector.tensor_add(
                        t4[:].rearrange("p (w i) -> p w i", i=4),
                        v[:, :, 0:4], v[:, :, 4:8])
                    t2 = tree_pool.tile([128, W * 2], bf16, tag="t2")
                    v = t4[:].rearrange("p (w i) -> p w i", i=4)
                    nc.vector.tensor_add(
                        t2[:].rearrange("p (w i) -> p w i", i=2),
                        v[:, :, 0:2], v[:, :, 2:4])
                    s1 = tree_pool.tile([128, W], f32, tag="s1")
                    v = t2[:].rearrange("p (w i) -> p w i", i=2)
                    nc.vector.scalar_tensor_tensor(
                        s1[:], v[:, :, 0:1], 1.0, v[:, :, 1:2],
                        op0=mybir.AluOpType.mult, op1=mybir.AluOpType.add)
                    nc.vector.scalar_tensor_tensor(
                        wlp[:, g * W:(g + 1) * W], s1[:], -1.0, lcs[:],
                        op0=mybir.AluOpType.mult, op1=mybir.AluOpType.add)

            # phase 2
            nc.scalar.activation(ebuf[:], wlp[:],
                                 mybir.ActivationFunctionType.Exp)
            rsum = const_pool.tile([128, N_TILES], f32)
            nc.vector.reduce_sum(
                rsum[:], ebuf[:].rearrange("p (t k) -> p t k", k=K),
                axis=mybir.AxisListType.X)
            lnr = const_pool.tile([128, N_TILES], f32)
            nc.scalar.activation(lnr[:], rsum[:],
                                 mybir.ActivationFunctionType.Ln)
            msum = const_pool.tile([128, N_TILES], f32)
            nc.vector.tensor_mul(msum[:], lnr[:], msks[:])
            csum = const_pool.tile([128, 1], f32)
            nc.vector.reduce_sum(csum[:], msum[:], axis=mybir.AxisListType.X)

            from concourse import bass_isa
            res = const_pool.tile([128, 1], f32)
            nc.gpsimd.partition_all_reduce(res[:], csum[:], channels=128,
                                           reduce_op=bass_isa.ReduceOp.add)
            nc.sync.dma_start(out[:], res[0:1, :])

    nc.compile()
    return nc


def _precompute(weights, means, covariances):
    """Host-side O(K d^3) prep in float64. Returns (bm, lconst_row, m0)."""
    import ml_dtypes

    K, d = means.shape
    L = np.linalg.cholesky(covariances.astype(np.float64))
    half_logdet = np.log(np.diagonal(L, axis1=-2, axis2=-1)).sum(-1)
    eye = np.eye(d)
    B = np.stack([np.linalg.solve(L[k], eye) for k in range(K)])  # L^-1
    B = B / np.sqrt(2.0)
    c = np.einsum('kij,kj->ki', B, means.astype(np.float64))      # B mu
    C = (np.log(weights.astype(np.float64))
         - 0.5 * d * np.log(2.0 * np.pi) - half_logdet)
    m0 = float(C.max()) - 20.0

    bm = np.zeros((CONTR, KD), np.float32)
    for k in range(K):
        bm[0:d, k * d:(k + 1) * d] = B[k].T.astype(np.float32)
        hi = np.asarray(-c[k], np.float32).astype(ml_dtypes.bfloat16)
        bm[d, k * d:(k + 1) * d] = hi.astype(np.float32)
        bm[d + 1, k * d:(k + 1) * d] = (-c[k] - hi.astype(np.float64)
                                        ).astype(np.float32)
    lconst_row = (C - m0).astype(np.float32)                      # [16]
    return bm.astype(ml_dtypes.bfloat16), lconst_row, m0


def _make_inputs(data, bm, lconst_row):
    """Build the 8 per-core input maps (transposed, padded, ones-rows)."""
    import ml_dtypes

    lconst = np.tile(lconst_row, GROUP_TILES)[None, :].repeat(128, 0)
    lconst = np.ascontiguousarray(lconst, np.float32)
    mask = np.zeros((128, N_TILES), np.float32)
    for t in range(N_TILES):
        v = min(max(PER_CORE - t * TILE_P, 0), TILE_P)
        mask[:v, t] = 1.0
    ones = np.ones((128, 1), np.float32)

    in_maps = []
    for cc in range(N_CORES):
        sl = data[cc * PER_CORE:(cc + 1) * PER_CORE]
        xpc = np.zeros((CONTR, PADDED), ml_dtypes.bfloat16)
        xpc[0:N_FEATURES, :PER_CORE] = sl.T.astype(ml_dtypes.bfloat16)
        xpc[N_FEATURES, :] = 1.0
        xpc[N_FEATURES + 1, :] = 1.0
        in_maps.append({"xp": xpc, "bm": bm, "lconst": lconst,
                        "mask": mask, "ones": ones})
    return in_maps


def _run(data, weights, means, covariances, trace=False):
    from concourse.bass_utils import run_bass_kernel_spmd

    data = np.asarray(data, np.float32)
    bm, lconst_row, m0 = _precompute(np.asarray(weights), np.asarray(means),
                                     np.asarray(covariances))
    if "nc" not in _CACHE:
        _CACHE["nc"] = _build_nc()
    nc = _CACHE["nc"]

    in_maps = _make_inputs(data, bm, lconst_row)
    res = run_bass_kernel_spmd(nc, in_maps, list(range(N_CORES)), trace=trace)
    total = 0.0
    for cc in range(N_CORES):
        total += float(res.results[cc]["out"][0, 0]) + PER_CORE * m0
    return np.float32(total), res


def kernel(data, weights, means, covariances):
    return _run(data, weights, means, covariances)[0]


# revision 8
# speedup vs baseline: 1.8167x; 1.1583x over previous
"""GMM log-likelihood kernel for Trainium2 (Bass/Tile), 8-core data-parallel.

Math (host precompute in f64):
  B' = L^{-1} / sqrt(2),  S_k(x) = ||B'_k x||^2 = 0.5 maha-quadratic part
  wlp_k(x) = -S_k(x) + w_k . x + (C_k - m0),  w_k = B^T B mu_k,
  C_k = log pi_k - d/2 log 2pi - half_logdet_k - 0.5 ||B mu_k||^2
  out = sum_x [ m0 + log sum_k exp(wlp_k(x)) ]

Per core (25000 samples, padded to 196 tiles of 128):
  The PE runs in 64x128 row-tiled mode: even data-tiles' x^T lives in SBUF
  partitions 0:64 and computes on array rows 0:64 (tile T0), odd tiles in
  partitions 64:128 on rows 64:128 (T8) -- the two matmul streams execute
  CONCURRENTLY (the 66-row contraction only half-fills the array, and the
  HAM clock stays at 1.2 GHz for this duty cycle, so packing two matmuls
  recovers the lost 2x).  Per tile: Y psum [128,1024] (2 banks, 3 bufs)
  + a 16-col lin matmul into a per-parity group bank; a DVE STT folds
  lin+const to SBUF.  Squares split between ACT (Square -> bf16) and a
  custom DVE op sq(a)+b fusing the 64->32 fold while reading PSUM; a bf16
  TT tree (2x DVE mode) does 32->2 per (tile,k) batched per 14-tile group.
  Phase 2: exp / k-reduce / ln / mask / reduce / gpsimd partition-fold.
Host sums the 8 per-core scalars (+ m0 per real sample).
"""

import numpy as np

N_COMPONENTS = 16
N_FEATURES = 64
N_SAMPLES = 200000
N_CORES = 8
PER_CORE = N_SAMPLES // N_CORES          # 25000
TILE_P = 128
N_TILES = -(-PER_CORE // TILE_P)         # 196 (ceil)
PADDED = N_TILES * TILE_P                # 25088
KD = N_COMPONENTS * N_FEATURES           # 1024
K = N_COMPONENTS

GROUP_TILES = 14                         # tiles per tree batch
N_GROUPS = N_TILES // GROUP_TILES        # 14
PAIRS_PER_GROUP = GROUP_TILES // 2       # 7
CHUNK_PAIRS = 14                         # DMA chunk = 2 groups
N_CHUNKS = N_TILES // (2 * CHUNK_PAIRS)  # 7

# tile index -> drain path: "H" tiles split the square between ACT (first
# half of each k-group) and the DVE custom op; "A" tiles let ACT square
# everything.  (ACT/DVE busy-balance knob.)
TILE_PERIOD = 9
H_TILE_MOD = (0, 2, 4, 6, 8)

_CACHE = {}


def _register_sq2():
    """Custom DVE op  out = sq(in0) + in1  (f32 internal): in0 = raw-Y half
    from PSUM (DVE may read only ONE PSUM operand), in1 = the ACT-squared
    other half from SBUF."""
    from concourse import dve_ops
    from concourse.dve_spec import Spec, Src0, Src1, sq, lower, _has_src1
    from concourse.dve_uop import DveOpSpec

    if any(op.name == "SQ1_ADD_ANT" for op in dve_ops.OPS):
        return next(op for op in dve_ops.OPS if op.name == "SQ1_ADD_ANT")
    spec = Spec(
        body=sq(Src0) + Src1,
        reference=lambda in0, in1, s0, s1, imm2: (
            in0.astype(np.float32) ** 2 + in1.astype(np.float32)),
    )
    shas = {}
    for ver in ("v3", "v4"):
        c = DveOpSpec(name="SQ1_ADD_ANT", opcode=17,
                      uops=lower(spec, ver=ver), rd1_en=_has_src1(spec))
        shas[ver] = c.sha(ver)
    op = dve_ops.DveOp("SQ1_ADD_ANT", spec, subdim=False, uops_sha=shas)
    row = max(dve_ops._SUB_OPCODE_FOR_NAME.values()) + 1
    assert row < 0x20
    dve_ops.OPS.append(op)
    dve_ops._SUB_OPCODE_FOR_NAME[op.name] = row
    dve_ops.CUSTOM_DVE_SPECS[op.name] = spec
    return op


def _build_nc():
    import concourse.tile as tile
    from concourse import bacc, mybir, bass_isa

    sq2 = _register_sq2()

    f32 = mybir.dt.float32
    bf16 = mybir.dt.bfloat16
    W = GROUP_TILES * K                  # 224 wlp columns per group
    HALF = N_TILES * TILE_P // 2         # 12544 columns of paired x^T

    nc = bacc.Bacc("TRN2", target_bir_lowering=False, debug=False,
                   num_devices=N_CORES)

    xp = nc.dram_tensor("xp", [128, HALF], bf16, kind="ExternalInput").ap()
    bm = nc.dram_tensor("bm", [128, KD + K], bf16, kind="ExternalInput").ap()
    lconst = nc.dram_tensor("lconst", [128, W], f32, kind="ExternalInput").ap()
    mask = nc.dram_tensor("mask", [128, N_TILES], f32, kind="ExternalInput").ap()
    out = nc.dram_tensor("out", [1, 1], f32, kind="ExternalOutput").ap()

    with tile.TileContext(nc) as tc:
        with (
            tc.tile_pool(name="const", bufs=1) as const_pool,
            tc.tile_pool(name="xin", bufs=2) as xin_pool,
            tc.tile_pool(name="ysq", bufs=3) as ysq_pool,
            tc.tile_pool(name="sq32", bufs=2) as sq32_pool,
            tc.tile_pool(name="tree", bufs=2) as tree_pool,
            tc.tile_pool(name="lin", bufs=2) as lin_pool,
            tc.tile_pool(name="wb", bufs=1) as wb_pool,
            tc.tile_pool(name="yp", bufs=3, space="PSUM") as yp_pool,
            tc.tile_pool(name="lpe", bufs=1, space="PSUM") as lpe_pool,
            tc.tile_pool(name="lpo", bufs=1, space="PSUM") as lpo_pool,
        ):
            bms = const_pool.tile([128, KD + K], bf16)
            nc.sync.dma_start(bms[:], bm[:])
            lcs = const_pool.tile([128, W], f32)
            nc.sync.dma_start(lcs[:], lconst[:])
            msks = const_pool.tile([128, N_TILES], f32)
            nc.sync.dma_start(msks[:], mask[:])

            wlp = wb_pool.tile([128, N_TILES * K], f32)
            ebuf = wb_pool.tile([128, N_TILES * K], f32)

            def drain(t_idx, yp, sq32, sqc):
                """Square+fold Y psum [128,1024] for one data-tile into
                sq32[:, sqc:sqc+512] (bf16, [k,32] layout)."""
                ypv = yp[:].rearrange("p (k i) -> p k i", i=64)
                if t_idx % TILE_PERIOD in H_TILE_MOD:
                    ysa = ysq_pool.tile([128, 512], bf16, tag="ysa")
                    nc.scalar.activation(
                        ysa[:].rearrange("p (k i) -> p k i", i=32),
                        ypv[:, :, 0:32],
                        mybir.ActivationFunctionType.Square)
                    nc.vector._custom_dve(
                        sq2,
                        out=sq32[:, sqc:sqc + 512]
                        .rearrange("p (k i) -> p k i", i=32),
                        in0=ypv[:, :, 32:64],
                        in1=ysa[:].rearrange("p (k i) -> p k i", i=32),
                    )
                else:
                    ysq = ysq_pool.tile([128, KD], bf16, tag="ysq")
                    nc.scalar.activation(
                        ysq[:], yp[:], mybir.ActivationFunctionType.Square)
                    yv = ysq[:].rearrange("p (k i) -> p k i", i=64)
                    nc.vector.tensor_add(
                        sq32[:, sqc:sqc + 512]
                        .rearrange("p (k i) -> p k i", i=32),
                        yv[:, :, 0:32], yv[:, :, 32:64])

            for c in range(N_CHUNKS):
                xb = xin_pool.tile([128, CHUNK_PAIRS * TILE_P], bf16, tag="xb")
                c0 = c * CHUNK_PAIRS * TILE_P
                nc.sync.dma_start(xb[:], xp[:, c0:c0 + CHUNK_PAIRS * TILE_P])

                for gl in range(2):              # two 14-tile groups per chunk
                    g = 2 * c + gl
                    sq32 = sq32_pool.tile([128, GROUP_TILES * 512], bf16,
                                          tag="sq32")
                    lpe = lpe_pool.tile([128, PAIRS_PER_GROUP * K], f32,
                                        tag="lpe")
                    lpo = lpo_pool.tile([128, PAIRS_PER_GROUP * K], f32,
                                        tag="lpo")
                    for pl in range(PAIRS_PER_GROUP):
                        xc = (gl * PAIRS_PER_GROUP + pl) * TILE_P
                        ype = yp_pool.tile([128, KD], f32, tag="yp")
                        ypo = yp_pool.tile([128, KD], f32, tag="yp")
                        lhsE = xb[0:64, xc:xc + TILE_P]
                        lhsO = xb[64:128, xc:xc + TILE_P]
                        nc.tensor.matmul(ype[:, 0:512], lhsE,
                                         bms[0:64, 0:512])
                        nc.tensor.matmul(ypo[:, 0:512], lhsO,
                                         bms[64:128, 0:512])
                        nc.tensor.matmul(ype[:, 512:1024], lhsE,
                                         bms[0:64, 512:1024])
                        nc.tensor.matmul(ypo[:, 512:1024], lhsO,
                                         bms[64:128, 512:1024])
                        nc.tensor.matmul(lpe[:, pl * K:(pl + 1) * K], lhsE,
                                         bms[0:64, KD:KD + K])
                        nc.tensor.matmul(lpo[:, pl * K:(pl + 1) * K], lhsO,
                                         bms[64:128, KD:KD + K])
                        t0 = g * GROUP_TILES + 2 * pl
                        drain(t0, ype, sq32, (2 * pl) * 512)
                        drain(t0 + 1, ypo, sq32, (2 * pl + 1) * 512)

                    # lin + const -> SBUF (frees the lp banks early)
                    linb = lin_pool.tile([128, W], f32, tag="linb")
                    lbv = linb[:].rearrange("p (t k) -> p t k", k=K)
                    lcv = lcs[:].rearrange("p (t k) -> p t k", k=K)
                    nc.vector.scalar_tensor_tensor(
                        lbv[:, 0:GROUP_TILES:2, :],
                        lpe[:].rearrange("p (t k) -> p t k", k=K),
                        1.0, lcv[:, 0:GROUP_TILES:2, :],
                        op0=mybir.AluOpType.mult, op1=mybir.AluOpType.add)
                    nc.vector.scalar_tensor_tensor(
                        lbv[:, 1:GROUP_TILES:2, :],
                        lpo[:].rearrange("p (t k) -> p t k", k=K),
                        1.0, lcv[:, 1:GROUP_TILES:2, :],
                        op0=mybir.AluOpType.mult, op1=mybir.AluOpType.add)

                    # tree: 32 -> 16 -> 8 -> 4 -> 2 (bf16 TT @2x), then STTs
                    t16 = tree_pool.tile([128, W * 16], bf16, tag="t16")
                    v = sq32[:].rearrange("p (w i) -> p w i", i=32)
                    nc.vector.tensor_add(
                        t16[:].rearrange("p (w i) -> p w i", i=16),
                        v[:, :, 0:16], v[:, :, 16:32])
                    t8 = tree_pool.tile([128, W * 8], bf16, tag="t8")
                    v = t16[:].rearrange("p (w i) -> p w i", i=16)
                    nc.vector.tensor_add(
                        t8[:].rearrange("p (w i) -> p w i", i=8),
                        v[:, :, 0:8], v[:, :, 8:16])
                    t4 = tree_pool.tile([128, W * 4], bf16, tag="t4")
                    v = t8[:].rearrange("p (w i) -> p w i", i=8)
                    nc.vector.tensor_add(
                        t4[:].rearrange("p (w i) -> p w i", i=4),
                        v[:, :, 0:4], v[:, :, 4:8])
                    t2 = tree_pool.tile([128, W * 2], bf16, tag="t2")
                    v = t4[:].rearrange("p (w i) -> p w i", i=4)
                    nc.vector.tensor_add(
                        t2[:].rearrange("p (w i) -> p w i", i=2),
                        v[:, :, 0:2], v[:, :, 2:4])
                    s1 = tree_pool.tile([128, W], f32, tag="s1")
                    v = t2[:].rearrange("p (w i) -> p w i", i=2)
                    nc.vector.scalar_tensor_tensor(
                        s1[:], v[:, :, 0:1], -1.0, v[:, :, 1:2],
                        op0=mybir.AluOpType.mult,
                        op1=mybir.AluOpType.subtract)
                    nc.vector.scalar_tensor_tensor(
                        wlp[:, g * W:(g + 1) * W], s1[:], 1.0, linb[:],
                        op0=mybir.AluOpType.mult, op1=mybir.AluOpType.add)

            # phase 2
            nc.scalar.activation(ebuf[:], wlp[:],
                                 mybir.ActivationFunctionType.Exp)
            rsum = const_pool.tile([128, N_TILES], f32)
            nc.vector.reduce_sum(
                rsum[:], ebuf[:].rearrange("p (t k) -> p t k", k=K),
                axis=mybir.AxisListType.X)
            lnr = const_pool.tile([128, N_TILES], f32)
            nc.scalar.activation(lnr[:], rsum[:],
                                 mybir.ActivationFunctionType.Ln)
            msum = const_pool.tile([128, N_TILES], f32)
            nc.vector.tensor_mul(msum[:], lnr[:], msks[:])
            csum = const_pool.tile([128, 1], f32)
            nc.vector.reduce_sum(csum[:], msum[:], axis=mybir.AxisListType.X)

            res = const_pool.tile([128, 1], f32)
            nc.gpsimd.partition_all_reduce(res[:], csum[:], channels=128,
                                           reduce_op=bass_isa.ReduceOp.add)
            nc.sync.dma_start(out[:], res[0:1, :])

    nc.compile()
    return nc


def _precompute(weights, means, covariances):
    """Host-side O(K d^3) prep in float64. Returns (bm, lconst_row, m0)."""
    import ml_dtypes

    Kc, d = means.shape
    L = np.linalg.cholesky(covariances.astype(np.float64))
    half_logdet = np.log(np.diagonal(L, axis1=-2, axis2=-1)).sum(-1)
    eye = np.eye(d)
    B = np.stack([np.linalg.solve(L[k], eye) for k in range(Kc)])  # L^-1
    mu = means.astype(np.float64)
    c = np.einsum('kij,kj->ki', B, mu)                # B mu
    w_lin = np.einsum('kij,ki->kj', B, c)             # B^T B mu
    r = (c * c).sum(-1)
    C = (np.log(weights.astype(np.float64))
         - 0.5 * d * np.log(2.0 * np.pi) - half_logdet - 0.5 * r)
    m0 = float(C.max()) - 20.0
    Bs = B / np.sqrt(2.0)                             # S = 0.5 ||B x||^2

    half = np.zeros((d, KD + Kc), np.float32)
    for k in range(Kc):
        half[:, k * d:(k + 1) * d] = Bs[k].T.astype(np.float32)
    half[:, KD:] = w_lin.T.astype(np.float32)
    bm = np.vstack([half, half]).astype(ml_dtypes.bfloat16)  # [128, 1040]
    lconst_row = (C - m0).astype(np.float32)                 # [16]
    return bm, lconst_row, m0


def _make_inputs(data, bm, lconst_row):
    """8 per-core input maps: x^T parity-split into top/bottom partitions."""
    import ml_dtypes

    lconst = np.tile(lconst_row, GROUP_TILES)[None, :].repeat(128, 0)
    lconst = np.ascontiguousarray(lconst, np.float32)
    mask = np.zeros((128, N_TILES), np.float32)
    for t in range(N_TILES):
        v = min(max(PER_CORE - t * TILE_P, 0), TILE_P)
        mask[:v, t] = 1.0

    in_maps = []
    for cc in range(N_CORES):
        sl = data[cc * PER_CORE:(cc + 1) * PER_CORE]
        xt = np.zeros((N_FEATURES, PADDED), np.float32)
        xt[:, :PER_CORE] = sl.T
        xt = xt.reshape(N_FEATURES, N_TILES // 2, 2, TILE_P)
        xpc = np.empty((128, PADDED // 2), np.float32)
        xpc[0:64] = xt[:, :, 0, :].reshape(N_FEATURES, -1)
        xpc[64:128] = xt[:, :, 1, :].reshape(N_FEATURES, -1)
        in_maps.append({"xp": xpc.astype(ml_dtypes.bfloat16), "bm": bm,
                        "lconst": lconst, "mask": mask})
    return in_maps


def _run(data, weights, means, covariances, trace=False):
    from concourse.bass_utils import run_bass_kernel_spmd

    data = np.asarray(data, np.float32)
    bm, lconst_row, m0 = _precompute(np.asarray(weights), np.asarray(means),
                                     np.asarray(covariances))
    if "nc" not in _CACHE:
        _CACHE["nc"] = _build_nc()
    nc = _CACHE["nc"]

    in_maps = _make_inputs(data, bm, lconst_row)
    res = run_bass_kernel_spmd(nc, in_maps, list(range(N_CORES)), trace=trace)
    total = 0.0
    for cc in range(N_CORES):
        total += float(res.results[cc]["out"][0, 0]) + PER_CORE * m0
    return np.float32(total), res


def kernel(data, weights, means, covariances):
    return _run(data, weights, means, covariances)[0]


# revision 11
# speedup vs baseline: 1.8395x; 1.0126x over previous
"""GMM log-likelihood kernel for Trainium2 (Bass/Tile), 8-core data-parallel.

Math (host precompute in f64):
  B' = L^{-1} / sqrt(2),  S_k(x) = ||B'_k x||^2 = 0.5 maha-quadratic part
  wlp_k(x) = -S_k(x) + w_k . x + (C_k - m0),  w_k = B^T B mu_k,
  C_k = log pi_k - d/2 log 2pi - half_logdet_k - 0.5 ||B mu_k||^2
  out = sum_x [ m0 + log sum_k exp(wlp_k(x)) ]

Per core (25000 samples, padded to 196 tiles of 128):
  The PE runs in 64x128 row-tiled mode: even data-tiles' x^T lives in SBUF
  partitions 0:64 and computes on array rows 0:64 (tile T0), odd tiles in
  partitions 64:128 on rows 64:128 (T8) -- the two matmul streams execute
  CONCURRENTLY (the 66-row contraction only half-fills the array, and the
  HAM clock stays at 1.2 GHz for this duty cycle, so packing two matmuls
  recovers the lost 2x).  Per tile: Y psum [128,1024] (2 banks, 3 bufs)
  + a 16-col lin matmul into a per-parity group bank; a DVE STT folds
  lin+const to SBUF.  Squares split between ACT (Square -> bf16) and a
  custom DVE op sq(a)+b fusing the 64->32 fold while reading PSUM; a bf16
  TT tree (2x DVE mode) does 32->2 per (tile,k) batched per 14-tile group.
  Phase 2: exp / k-reduce / ln / mask / reduce / gpsimd partition-fold.
Host sums the 8 per-core scalars (+ m0 per real sample).
"""

import numpy as np

N_COMPONENTS = 16
N_FEATURES = 64
N_SAMPLES = 200000
N_CORES = 8
PER_CORE = N_SAMPLES // N_CORES          # 25000
TILE_P = 128
N_TILES = -(-PER_CORE // TILE_P)         # 196 (ceil)
PADDED = N_TILES * TILE_P                # 25088
KD = N_COMPONENTS * N_FEATURES           # 1024
K = N_COMPONENTS

GROUP_TILES = 14                         # tiles per tree batch
N_GROUPS = N_TILES // GROUP_TILES        # 14
PAIRS_PER_GROUP = GROUP_TILES // 2       # 7
CHUNK_PAIRS = 14                         # DMA chunk = 2 groups
N_CHUNKS = N_TILES // (2 * CHUNK_PAIRS)  # 7

# tile index -> drain path: "H" tiles split the square between ACT (first
# half of each k-group) and the DVE custom op; "A" tiles let ACT square
# everything.  (ACT/DVE busy-balance knob.)
TILE_PERIOD = 9
H_TILE_MOD = (0, 2, 4, 6)

_CACHE = {}


def _register_sq2():
    """Custom DVE op  out = sq(in0) + in1  (f32 internal): in0 = raw-Y half
    from PSUM (DVE may read only ONE PSUM operand), in1 = the ACT-squared
    other half from SBUF."""
    from concourse import dve_ops
    from concourse.dve_spec import Spec, Src0, Src1, sq, lower, _has_src1
    from concourse.dve_uop import DveOpSpec

    if any(op.name == "SQ1_ADD_ANT" for op in dve_ops.OPS):
        return next(op for op in dve_ops.OPS if op.name == "SQ1_ADD_ANT")
    spec = Spec(
        body=sq(Src0) + Src1,
        reference=lambda in0, in1, s0, s1, imm2: (
            in0.astype(np.float32) ** 2 + in1.astype(np.float32)),
    )
    shas = {}
    for ver in ("v3", "v4"):
        c = DveOpSpec(name="SQ1_ADD_ANT", opcode=17,
                      uops=lower(spec, ver=ver), rd1_en=_has_src1(spec))
        shas[ver] = c.sha(ver)
    op = dve_ops.DveOp("SQ1_ADD_ANT", spec, subdim=False, uops_sha=shas)
    row = max(dve_ops._SUB_OPCODE_FOR_NAME.values()) + 1
    assert row < 0x20
    dve_ops.OPS.append(op)
    dve_ops._SUB_OPCODE_FOR_NAME[op.name] = row
    dve_ops.CUSTOM_DVE_SPECS[op.name] = spec
    return op


def _build_nc():
    import concourse.tile as tile
    from concourse import bacc, mybir, bass_isa

    sq2 = _register_sq2()

    f32 = mybir.dt.float32
    bf16 = mybir.dt.bfloat16
    W = GROUP_TILES * K                  # 224 wlp columns per group
    HALF = N_TILES * TILE_P // 2         # 12544 columns of paired x^T

    nc = bacc.Bacc("TRN2", target_bir_lowering=False, debug=False,
                   num_devices=N_CORES)

    xp = nc.dram_tensor("xp", [128, HALF], bf16, kind="ExternalInput").ap()
    bm = nc.dram_tensor("bm", [128, KD + K], bf16, kind="ExternalInput").ap()
    lconst = nc.dram_tensor("lconst", [128, W], f32, kind="ExternalInput").ap()
    mask = nc.dram_tensor("mask", [128, N_TILES], f32, kind="ExternalInput").ap()
    out = nc.dram_tensor("out", [1, 1], f32, kind="ExternalOutput").ap()

    with tile.TileContext(nc) as tc:
        with (
            tc.tile_pool(name="const", bufs=1) as const_pool,
            tc.tile_pool(name="xin", bufs=2) as xin_pool,
            tc.tile_pool(name="ysq", bufs=4) as ysq_pool,
            tc.tile_pool(name="sq32", bufs=2) as sq32_pool,
            tc.tile_pool(name="tree", bufs=2) as tree_pool,
            tc.tile_pool(name="lin", bufs=2) as lin_pool,
            tc.tile_pool(name="wb", bufs=1) as wb_pool,
            tc.tile_pool(name="yp", bufs=3, space="PSUM") as yp_pool,
            tc.tile_pool(name="lpe", bufs=1, space="PSUM") as lpe_pool,
            tc.tile_pool(name="lpo", bufs=1, space="PSUM") as lpo_pool,
        ):
            bms = const_pool.tile([128, KD + K], bf16)
            nc.sync.dma_start(bms[:], bm[:])
            lcs = const_pool.tile([128, W], f32)
            nc.sync.dma_start(lcs[:], lconst[:])
            msks = const_pool.tile([128, N_TILES], f32)
            nc.sync.dma_start(msks[:], mask[:])

            wlp = wb_pool.tile([128, N_TILES * K], f32)
            ebuf = wb_pool.tile([128, N_TILES * K], f32)

            def drain(t_idx, yp, sq32, sqc):
                """Square+fold Y psum [128,1024] for one data-tile into
                sq32[:, sqc:sqc+512] (bf16, [k,32] layout)."""
                ypv = yp[:].rearrange("p (k i) -> p k i", i=64)
                if t_idx % TILE_PERIOD in H_TILE_MOD:
                    ysa = ysq_pool.tile([128, 512], bf16, tag="ysa")
                    nc.scalar.activation(
                        ysa[:].rearrange("p (k i) -> p k i", i=32),
                        ypv[:, :, 0:32],
                        mybir.ActivationFunctionType.Square)
                    nc.vector._custom_dve(
                        sq2,
                        out=sq32[:, sqc:sqc + 512]
                        .rearrange("p (k i) -> p k i", i=32),
                        in0=ypv[:, :, 32:64],
                        in1=ysa[:].rearrange("p (k i) -> p k i", i=32),
                    )
                else:
                    ysq = ysq_pool.tile([128, KD], bf16, tag="ysq")
                    nc.scalar.activation(
                        ysq[:], yp[:], mybir.ActivationFunctionType.Square)
                    yv = ysq[:].rearrange("p (k i) -> p k i", i=64)
                    nc.vector.tensor_add(
                        sq32[:, sqc:sqc + 512]
                        .rearrange("p (k i) -> p k i", i=32),
                        yv[:, :, 0:32], yv[:, :, 32:64])

            def make_tree(g, sq32, linb):
                """Deferred tree emitter: 32 -> 2 (bf16 TT @2x), then STTs."""
                def emit():
                    t16 = tree_pool.tile([128, W * 16], bf16, tag="t16")
                    v = sq32[:].rearrange("p (w i) -> p w i", i=32)
                    nc.vector.tensor_add(
                        t16[:].rearrange("p (w i) -> p w i", i=16),
                        v[:, :, 0:16], v[:, :, 16:32])
                    t8 = tree_pool.tile([128, W * 8], bf16, tag="t8")
                    v = t16[:].rearrange("p (w i) -> p w i", i=16)
                    nc.vector.tensor_add(
                        t8[:].rearrange("p (w i) -> p w i", i=8),
                        v[:, :, 0:8], v[:, :, 8:16])
                    t4 = tree_pool.tile([128, W * 4], bf16, tag="t4")
                    v = t8[:].rearrange("p (w i) -> p w i", i=8)
                    nc.vector.tensor_add(
                        t4[:].rearrange("p (w i) -> p w i", i=4),
                        v[:, :, 0:4], v[:, :, 4:8])
                    t2 = tree_pool.tile([128, W * 2], bf16, tag="t2")
                    v = t4[:].rearrange("p (w i) -> p w i", i=4)
                    nc.vector.tensor_add(
                        t2[:].rearrange("p (w i) -> p w i", i=2),
                        v[:, :, 0:2], v[:, :, 2:4])
                    s1 = tree_pool.tile([128, W], f32, tag="s1")
                    v = t2[:].rearrange("p (w i) -> p w i", i=2)
                    nc.vector.scalar_tensor_tensor(
                        s1[:], v[:, :, 0:1], -1.0, v[:, :, 1:2],
                        op0=mybir.AluOpType.mult,
                        op1=mybir.AluOpType.subtract)
                    nc.vector.scalar_tensor_tensor(
                        wlp[:, g * W:(g + 1) * W], s1[:], 1.0, linb[:],
                        op0=mybir.AluOpType.mult, op1=mybir.AluOpType.add)
                return emit

            pending_tree = None
            for c in range(N_CHUNKS):
                xb = xin_pool.tile([128, CHUNK_PAIRS * TILE_P], bf16, tag="xb")
                c0 = c * CHUNK_PAIRS * TILE_P
                nc.sync.dma_start(xb[:], xp[:, c0:c0 + CHUNK_PAIRS * TILE_P])

                for gl in range(2):              # two 14-tile groups per chunk
                    g = 2 * c + gl
                    sq32 = sq32_pool.tile([128, GROUP_TILES * 512], bf16,
                                          tag="sq32")
                    lpe = lpe_pool.tile([128, PAIRS_PER_GROUP * K], f32,
                                        tag="lpe")
                    lpo = lpo_pool.tile([128, PAIRS_PER_GROUP * K], f32,
                                        tag="lpo")
                    for pl in range(PAIRS_PER_GROUP):
                        xc = (gl * PAIRS_PER_GROUP + pl) * TILE_P
                        ype = yp_pool.tile([128, KD], f32, tag="yp")
                        ypo = yp_pool.tile([128, KD], f32, tag="yp")
                        lhsE = xb[0:64, xc:xc + TILE_P]
                        lhsO = xb[64:128, xc:xc + TILE_P]
                        nc.tensor.matmul(ype[:, 0:512], lhsE,
                                         bms[0:64, 0:512])
                        nc.tensor.matmul(ypo[:, 0:512], lhsO,
                                         bms[64:128, 0:512])
                        nc.tensor.matmul(ype[:, 512:1024], lhsE,
                                         bms[0:64, 512:1024])
                        nc.tensor.matmul(ypo[:, 512:1024], lhsO,
                                         bms[64:128, 512:1024])
                        nc.tensor.matmul(lpe[:, pl * K:(pl + 1) * K], lhsE,
                                         bms[0:64, KD:KD + K])
                        nc.tensor.matmul(lpo[:, pl * K:(pl + 1) * K], lhsO,
                                         bms[64:128, KD:KD + K])
                        t0 = g * GROUP_TILES + 2 * pl
                        drain(t0, ype, sq32, (2 * pl) * 512)
                        drain(t0 + 1, ypo, sq32, (2 * pl + 1) * 512)
                        if pl == 1 and pending_tree is not None:
                            # previous group's tree, emitted mid-stream so it
                            # doesn't head-block this group's per-tile DVE ops
                            pending_tree()
                            pending_tree = None

                    # lin + const -> SBUF (frees the lp banks early)
                    linb = lin_pool.tile([128, W], f32, tag="linb")
                    lbv = linb[:].rearrange("p (t k) -> p t k", k=K)
                    lcv = lcs[:].rearrange("p (t k) -> p t k", k=K)
                    nc.vector.scalar_tensor_tensor(
                        lbv[:, 0:GROUP_TILES:2, :],
                        lpe[:].rearrange("p (t k) -> p t k", k=K),
                        1.0, lcv[:, 0:GROUP_TILES:2, :],
                        op0=mybir.AluOpType.mult, op1=mybir.AluOpType.add)
                    nc.vector.scalar_tensor_tensor(
                        lbv[:, 1:GROUP_TILES:2, :],
                        lpo[:].rearrange("p (t k) -> p t k", k=K),
                        1.0, lcv[:, 1:GROUP_TILES:2, :],
                        op0=mybir.AluOpType.mult, op1=mybir.AluOpType.add)
                    pending_tree = make_tree(g, sq32, linb)
            pending_tree()

            # phase 2
            nc.scalar.activation(ebuf[:], wlp[:],
                                 mybir.ActivationFunctionType.Exp)
            rsum = const_pool.tile([128, N_TILES], f32)
            nc.vector.reduce_sum(
                rsum[:], ebuf[:].rearrange("p (t k) -> p t k", k=K),
                axis=mybir.AxisListType.X)
            lnr = const_pool.tile([128, N_TILES], f32)
            nc.scalar.activation(lnr[:], rsum[:],
                                 mybir.ActivationFunctionType.Ln)
            msum = const_pool.tile([128, N_TILES], f32)
            nc.vector.tensor_mul(msum[:], lnr[:], msks[:])
            csum = const_pool.tile([128, 1], f32)
            nc.vector.reduce_sum(csum[:], msum[:], axis=mybir.AxisListType.X)

            res = const_pool.tile([128, 1], f32)
            nc.gpsimd.partition_all_reduce(res[:], csum[:], channels=128,
                                           reduce_op=bass_isa.ReduceOp.add)
            nc.sync.dma_start(out[:], res[0:1, :])

    nc.compile()
    return nc


def _precompute(weights, means, covariances):
    """Host-side O(K d^3) prep in float64. Returns (bm, lconst_row, m0)."""
    import ml_dtypes

    Kc, d = means.shape
    L = np.linalg.cholesky(covariances.astype(np.float64))
    half_logdet = np.log(np.diagonal(L, axis1=-2, axis2=-1)).sum(-1)
    eye = np.eye(d)
    B = np.stack([np.linalg.solve(L[k], eye) for k in range(Kc)])  # L^-1
    mu = means.astype(np.float64)
    c = np.einsum('kij,kj->ki', B, mu)                # B mu
    w_lin = np.einsum('kij,ki->kj', B, c)             # B^T B mu
    r = (c * c).sum(-1)
    C = (np.log(weights.astype(np.float64))
         - 0.5 * d * np.log(2.0 * np.pi) - half_logdet - 0.5 * r)
    m0 = float(C.max()) - 20.0
    Bs = B / np.sqrt(2.0)                             # S = 0.5 ||B x||^2

    half = np.zeros((d, KD + Kc), np.float32)
    for k in range(Kc):
        half[:, k * d:(k + 1) * d] = Bs[k].T.astype(np.float32)
    half[:, KD:] = w_lin.T.astype(np.float32)
    bm = np.vstack([half, half]).astype(ml_dtypes.bfloat16)  # [128, 1040]
    lconst_row = (C - m0).astype(np.float32)                 # [16]
    return bm, lconst_row, m0


def _make_inputs(data, bm, lconst_row):
    """8 per-core input maps: x^T parity-split into top/bottom partitions."""
    import ml_dtypes

    lconst = np.tile(lconst_row, GROUP_TILES)[None, :].repeat(128, 0)
    lconst = np.ascontiguousarray(lconst, np.float32)
    mask = np.zeros((128, N_TILES), np.float32)
    for t in range(N_TILES):
        v = min(max(PER_CORE - t * TILE_P, 0), TILE_P)
        mask[:v, t] = 1.0

    in_maps = []
    for cc in range(N_CORES):
        sl = data[cc * PER_CORE:(cc + 1) * PER_CORE]
        xt = np.zeros((N_FEATURES, PADDED), np.float32)
        xt[:, :PER_CORE] = sl.T
        xt = xt.reshape(N_FEATURES, N_TILES // 2, 2, TILE_P)
        xpc = np.empty((128, PADDED // 2), np.float32)
        xpc[0:64] = xt[:, :, 0, :].reshape(N_FEATURES, -1)
        xpc[64:128] = xt[:, :, 1, :].reshape(N_FEATURES, -1)
        in_maps.append({"xp": xpc.astype(ml_dtypes.bfloat16), "bm": bm,
                        "lconst": lconst, "mask": mask})
    return in_maps


def _run(data, weights, means, covariances, trace=False):
    from concourse.bass_utils import run_bass_kernel_spmd

    data = np.asarray(data, np.float32)
    bm, lconst_row, m0 = _precompute(np.asarray(weights), np.asarray(means),
                                     np.asarray(covariances))
    if "nc" not in _CACHE:
        _CACHE["nc"] = _build_nc()
    nc = _CACHE["nc"]

    in_maps = _make_inputs(data, bm, lconst_row)
    res = run_bass_kernel_spmd(nc, in_maps, list(range(N_CORES)), trace=trace)
    total = 0.0
    for cc in range(N_CORES):
        total += float(res.results[cc]["out"][0, 0]) + PER_CORE * m0
    return np.float32(total), res


def kernel(data, weights, means, covariances):
    return _run(data, weights, means, covariances)[0]


# revision 21
# speedup vs baseline: 1.9615x; 1.0663x over previous
"""GMM log-likelihood kernel for Trainium2 (Bass/Tile), 8-core data-parallel.

Math (host precompute in f64):
  B' = L^{-1} / sqrt(2),  S_k(x) = ||B'_k x||^2 = 0.5 maha-quadratic part
  wlp_k(x) = -S_k(x) + w_k . x + (C_k - m0),  w_k = B^T B mu_k,
  C_k = log pi_k - d/2 log 2pi - half_logdet_k - 0.5 ||B mu_k||^2
  out = sum_x [ m0 + log sum_k exp(wlp_k(x)) ]

Per core (25000 samples, padded to 196 tiles of 128):
  The PE runs in 64x128 row-tiled mode: even data-tiles' x^T lives in SBUF
  partitions 0:64 and computes on array rows 0:64 (tile T0), odd tiles in
  partitions 64:128 on rows 64:128 (T8) -- the two matmul streams execute
  CONCURRENTLY (the 66-row contraction only half-fills the array, and the
  HAM clock stays at 1.2 GHz for this duty cycle, so packing two matmuls
  recovers the lost 2x).  Per tile: Y psum [128,1024] (2 banks, 3 bufs)
  + a 16-col lin matmul into a per-parity group bank; a DVE STT folds
  lin+const to SBUF.  Squares split between ACT (Square -> bf16) and a
  custom DVE op sq(a)+b fusing the 64->32 fold while reading PSUM; a bf16
  TT tree (2x DVE mode) does 32->2 per (tile,k) batched per 14-tile group.
  Phase 2: exp / k-reduce / ln / mask / reduce / gpsimd partition-fold.
Host sums the 8 per-core scalars (+ m0 per real sample).
"""

import numpy as np

N_COMPONENTS = 16
N_FEATURES = 64
N_SAMPLES = 200000
N_CORES = 8
PER_CORE = N_SAMPLES // N_CORES          # 25000
TILE_P = 128
N_TILES = -(-PER_CORE // TILE_P)         # 196 (ceil)
PADDED = N_TILES * TILE_P                # 25088
KD = N_COMPONENTS * N_FEATURES           # 1024
K = N_COMPONENTS

GROUP_TILES = 14                         # tiles per tree batch
N_GROUPS = N_TILES // GROUP_TILES        # 14
PAIRS_PER_GROUP = GROUP_TILES // 2       # 7
CHUNK_PAIRS = 14                         # DMA chunk = 2 groups
N_CHUNKS = N_TILES // (2 * CHUNK_PAIRS)  # 7

# pair index -> drain path: "H" pairs split each tile's square between ACT
# (first half of each k-group) and the DVE custom op; "A" pairs let ACT
# square everything and DVE fold both tiles in one batched TT.
# (ACT/DVE busy-balance knob.)
PAIR_PERIOD = 15
H_PAIR_MOD = (0, 2, 4, 6, 8, 10, 12)

_CACHE = {}


def _register_sq2():
    """Custom DVE op  out = sq(in0) + in1  (f32 internal): in0 = raw-Y half
    from PSUM (DVE may read only ONE PSUM operand), in1 = the ACT-squared
    other half from SBUF."""
    from concourse import dve_ops
    from concourse.dve_spec import Spec, Src0, Src1, sq, lower, _has_src1
    from concourse.dve_uop import DveOpSpec

    if any(op.name == "SQ1_ADD_ANT" for op in dve_ops.OPS):
        return next(op for op in dve_ops.OPS if op.name == "SQ1_ADD_ANT")
    spec = Spec(
        body=sq(Src0) + Src1,
        reference=lambda in0, in1, s0, s1, imm2: (
            in0.astype(np.float32) ** 2 + in1.astype(np.float32)),
    )
    shas = {}
    for ver in ("v3", "v4"):
        c = DveOpSpec(name="SQ1_ADD_ANT", opcode=17,
                      uops=lower(spec, ver=ver), rd1_en=_has_src1(spec))
        shas[ver] = c.sha(ver)
    op = dve_ops.DveOp("SQ1_ADD_ANT", spec, subdim=False, uops_sha=shas)
    row = max(dve_ops._SUB_OPCODE_FOR_NAME.values()) + 1
    assert row < 0x20
    dve_ops.OPS.append(op)
    dve_ops._SUB_OPCODE_FOR_NAME[op.name] = row
    dve_ops.CUSTOM_DVE_SPECS[op.name] = spec
    return op


def _build_nc():
    import concourse.tile as tile
    from concourse import bacc, mybir, bass_isa

    sq2 = _register_sq2()

    f32 = mybir.dt.float32
    bf16 = mybir.dt.bfloat16
    W = GROUP_TILES * K                  # 224 wlp columns per group
    HALF = N_TILES * TILE_P // 2         # 12544 columns of paired x^T

    nc = bacc.Bacc("TRN2", target_bir_lowering=False, debug=False,
                   num_devices=N_CORES)

    xp = nc.dram_tensor("xp", [128, HALF], bf16, kind="ExternalInput").ap()
    bm = nc.dram_tensor("bm", [128, KD + K], bf16, kind="ExternalInput").ap()
    lconst = nc.dram_tensor("lconst", [128, W], f32, kind="ExternalInput").ap()
    mask = nc.dram_tensor("mask", [128, N_TILES], f32, kind="ExternalInput").ap()
    out = nc.dram_tensor("out", [1, 1], f32, kind="ExternalOutput").ap()

    with tile.TileContext(nc) as tc:
        with (
            tc.tile_pool(name="const", bufs=1) as const_pool,
            tc.tile_pool(name="xin", bufs=2) as xin_pool,
            tc.tile_pool(name="ysq", bufs=4) as ysq_pool,
            tc.tile_pool(name="sq32", bufs=2) as sq32_pool,
            tc.tile_pool(name="tree", bufs=2) as tree_pool,
            tc.tile_pool(name="lin", bufs=2) as lin_pool,
            tc.tile_pool(name="wb", bufs=1) as wb_pool,
            tc.tile_pool(name="yp", bufs=3, space="PSUM") as yp_pool,
            tc.tile_pool(name="lpe", bufs=1, space="PSUM") as lpe_pool,
            tc.tile_pool(name="lpo", bufs=1, space="PSUM") as lpo_pool,
        ):
            bms = const_pool.tile([128, KD + K], bf16)
            nc.sync.dma_start(bms[:], bm[:])
            lcs = const_pool.tile([128, W], f32)
            msks = const_pool.tile([128, N_TILES], f32)

            wlp = wb_pool.tile([128, N_TILES * K], f32)
            ebuf = wb_pool.tile([128, N_TILES * K], f32)
            rsum = wb_pool.tile([128, N_TILES], f32)

            def drain_h(yp, sq32, sqc):
                """H path: ACT squares the 0:32 halves, DVE custom squares
                32:64 from PSUM and adds, emitting sq32[:, sqc:sqc+512]."""
                ypv = yp[:].rearrange("p (k i) -> p k i", i=64)
                ysa = ysq_pool.tile([128, 512], bf16, tag="ysa")
                nc.scalar.activation(
                    ysa[:].rearrange("p (k i) -> p k i", i=32),
                    ypv[:, :, 0:32],
                    mybir.ActivationFunctionType.Square)
                nc.vector._custom_dve(
                    sq2,
                    out=sq32[:, sqc:sqc + 512]
                    .rearrange("p (k i) -> p k i", i=32),
                    in0=ypv[:, :, 32:64],
                    in1=ysa[:].rearrange("p (k i) -> p k i", i=32),
                )

            def drain_a(ype, ypo, sq32, sqc):
                """A path: ACT squares both tiles -> one [128,2048] bf16
                buffer, one 2x-mode TT folds 64->32 for the whole pair."""
                ysq = ysq_pool.tile([128, 2 * KD], bf16, tag="ysq")
                nc.scalar.activation(
                    ysq[:, 0:KD], ype[:], mybir.ActivationFunctionType.Square)
                nc.scalar.activation(
                    ysq[:, KD:2 * KD], ypo[:],
                    mybir.ActivationFunctionType.Square)
                yv = ysq[:].rearrange("p (k i) -> p k i", i=64)
                nc.vector.tensor_add(
                    sq32[:, sqc:sqc + 1024]
                    .rearrange("p (k i) -> p k i", i=32),
                    yv[:, :, 0:32], yv[:, :, 32:64])

            def make_tree(g, sq32, linb):
                """Deferred emitter: tree 32 -> 2 (bf16 TT @2x), STTs,
                then this group's exp + k-reduce (keeps the end tail short;
                Exp shares the ACT table set with Square, so no reloads)."""
                def emit():
                    t16 = tree_pool.tile([128, W * 16], bf16, tag="t16")
                    v = sq32[:].rearrange("p (w i) -> p w i", i=32)
                    nc.vector.tensor_add(
                        t16[:].rearrange("p (w i) -> p w i", i=16),
                        v[:, :, 0:16], v[:, :, 16:32])
                    t8 = tree_pool.tile([128, W * 8], bf16, tag="t8")
                    v = t16[:].rearrange("p (w i) -> p w i", i=16)
                    nc.vector.tensor_add(
                        t8[:].rearrange("p (w i) -> p w i", i=8),
                        v[:, :, 0:8], v[:, :, 8:16])
                    t4 = tree_pool.tile([128, W * 4], bf16, tag="t4")
                    v = t8[:].rearrange("p (w i) -> p w i", i=8)
                    nc.vector.tensor_add(
                        t4[:].rearrange("p (w i) -> p w i", i=4),
                        v[:, :, 0:4], v[:, :, 4:8])
                    t2 = tree_pool.tile([128, W * 2], bf16, tag="t2")
                    v = t4[:].rearrange("p (w i) -> p w i", i=4)
                    nc.vector.tensor_add(
                        t2[:].rearrange("p (w i) -> p w i", i=2),
                        v[:, :, 0:2], v[:, :, 2:4])
                    s1 = tree_pool.tile([128, W], f32, tag="s1")
                    v = t2[:].rearrange("p (w i) -> p w i", i=2)
                    nc.vector.scalar_tensor_tensor(
                        s1[:], v[:, :, 0:1], -1.0, v[:, :, 1:2],
                        op0=mybir.AluOpType.mult,
                        op1=mybir.AluOpType.subtract)
                    nc.vector.scalar_tensor_tensor(
                        wlp[:, g * W:(g + 1) * W], s1[:], 1.0, linb[:],
                        op0=mybir.AluOpType.mult, op1=mybir.AluOpType.add)
                    nc.scalar.activation(
                        ebuf[:, g * W:(g + 1) * W], wlp[:, g * W:(g + 1) * W],
                        mybir.ActivationFunctionType.Exp)
                    nc.vector.reduce_sum(
                        rsum[:, g * GROUP_TILES:(g + 1) * GROUP_TILES],
                        ebuf[:, g * W:(g + 1) * W]
                        .rearrange("p (t k) -> p t k", k=K),
                        axis=mybir.AxisListType.X)
                return emit

            pending_tree = None
            for c in range(N_CHUNKS):
                xb = xin_pool.tile([128, CHUNK_PAIRS * TILE_P], bf16, tag="xb")
                c0 = c * CHUNK_PAIRS * TILE_P
                nc.sync.dma_start(xb[:], xp[:, c0:c0 + CHUNK_PAIRS * TILE_P])
                if c == 0:
                    # late-needed constants AFTER the first data chunk, so the
                    # first matmuls aren't queued behind them on the DMA ring
                    nc.sync.dma_start(lcs[:], lconst[:])
                    nc.sync.dma_start(msks[:], mask[:])

                for gl in range(2):              # two 14-tile groups per chunk
                    g = 2 * c + gl
                    sq32 = sq32_pool.tile([128, GROUP_TILES * 512], bf16,
                                          tag="sq32")
                    lpe = lpe_pool.tile([128, PAIRS_PER_GROUP * K], f32,
                                        tag="lpe")
                    lpo = lpo_pool.tile([128, PAIRS_PER_GROUP * K], f32,
                                        tag="lpo")
                    for pl in range(PAIRS_PER_GROUP):
                        xc = (gl * PAIRS_PER_GROUP + pl) * TILE_P
                        ype = yp_pool.tile([128, KD], f32, tag="yp")
                        ypo = yp_pool.tile([128, KD], f32, tag="yp")
                        lhsE = xb[0:64, xc:xc + TILE_P]
                        lhsO = xb[64:128, xc:xc + TILE_P]
                        nc.tensor.matmul(ype[:, 0:512], lhsE,
                                         bms[0:64, 0:512])
                        nc.tensor.matmul(ypo[:, 0:512], lhsO,
                                         bms[64:128, 0:512])
                        nc.tensor.matmul(ype[:, 512:1024], lhsE,
                                         bms[0:64, 512:1024])
                        nc.tensor.matmul(ypo[:, 512:1024], lhsO,
                                         bms[64:128, 512:1024])
                        nc.tensor.matmul(lpe[:, pl * K:(pl + 1) * K], lhsE,
                                         bms[0:64, KD:KD + K])
                        nc.tensor.matmul(lpo[:, pl * K:(pl + 1) * K], lhsO,
                                         bms[64:128, KD:KD + K])
                        pair = g * PAIRS_PER_GROUP + pl
                        if pair % PAIR_PERIOD in H_PAIR_MOD:
                            drain_h(ype, sq32, (2 * pl) * 512)
                            drain_h(ypo, sq32, (2 * pl + 1) * 512)
                        else:
                            drain_a(ype, ypo, sq32, (2 * pl) * 512)
                        if pl == 1 and pending_tree is not None:
                            # previous group's tree, emitted mid-stream so it
                            # doesn't head-block this group's per-tile DVE ops
                            pending_tree()
                            pending_tree = None

                    # lin + const -> SBUF (frees the lp banks early)
                    linb = lin_pool.tile([128, W], f32, tag="linb")
                    lbv = linb[:].rearrange("p (t k) -> p t k", k=K)
                    lcv = lcs[:].rearrange("p (t k) -> p t k", k=K)
                    nc.vector.scalar_tensor_tensor(
                        lbv[:, 0:GROUP_TILES:2, :],
                        lpe[:].rearrange("p (t k) -> p t k", k=K),
                        1.0, lcv[:, 0:GROUP_TILES:2, :],
                        op0=mybir.AluOpType.mult, op1=mybir.AluOpType.add)
                    nc.vector.scalar_tensor_tensor(
                        lbv[:, 1:GROUP_TILES:2, :],
                        lpo[:].rearrange("p (t k) -> p t k", k=K),
                        1.0, lcv[:, 1:GROUP_TILES:2, :],
                        op0=mybir.AluOpType.mult, op1=mybir.AluOpType.add)
                    pending_tree = make_tree(g, sq32, linb)
            pending_tree()

            # phase 2 (exp/k-reduce already done per group)
            lnr = const_pool.tile([128, N_TILES], f32)
            nc.scalar.activation(lnr[:], rsum[:],
                                 mybir.ActivationFunctionType.Ln)
            msum = const_pool.tile([128, N_TILES], f32)
            nc.vector.tensor_mul(msum[:], lnr[:], msks[:])
            csum = const_pool.tile([128, 1], f32)
            nc.vector.reduce_sum(csum[:], msum[:], axis=mybir.AxisListType.X)

            res = const_pool.tile([128, 1], f32)
            nc.gpsimd.partition_all_reduce(res[:], csum[:], channels=128,
                                           reduce_op=bass_isa.ReduceOp.add)
            nc.sync.dma_start(out[:], res[0:1, :])

    nc.compile()
    return nc


def _precompute(weights, means, covariances):
    """Host-side O(K d^3) prep in float64. Returns (bm, lconst_row, m0)."""
    import ml_dtypes

    Kc, d = means.shape
    L = np.linalg.cholesky(covariances.astype(np.float64))
    half_logdet = np.log(np.diagonal(L, axis1=-2, axis2=-1)).sum(-1)
    eye = np.eye(d)
    B = np.stack([np.linalg.solve(L[k], eye) for k in range(Kc)])  # L^-1
    mu = means.astype(np.float64)
    c = np.einsum('kij,kj->ki', B, mu)                # B mu
    w_lin = np.einsum('kij,ki->kj', B, c)             # B^T B mu
    r = (c * c).sum(-1)
    C = (np.log(weights.astype(np.float64))
         - 0.5 * d * np.log(2.0 * np.pi) - half_logdet - 0.5 * r)
    m0 = float(C.max()) - 20.0
    Bs = B / np.sqrt(2.0)                             # S = 0.5 ||B x||^2

    half = np.zeros((d, KD + Kc), np.float32)
    for k in range(Kc):
        half[:, k * d:(k + 1) * d] = Bs[k].T.astype(np.float32)
    half[:, KD:] = w_lin.T.astype(np.float32)
    bm = np.vstack([half, half]).astype(ml_dtypes.bfloat16)  # [128, 1040]
    lconst_row = (C - m0).astype(np.float32)                 # [16]
    return bm, lconst_row, m0


def _make_inputs(data, bm, lconst_row):
    """8 per-core input maps: x^T parity-split into top/bottom partitions."""
    import ml_dtypes

    lconst = np.tile(lconst_row, GROUP_TILES)[None, :].repeat(128, 0)
    lconst = np.ascontiguousarray(lconst, np.float32)
    mask = np.zeros((128, N_TILES), np.float32)
    for t in range(N_TILES):
        v = min(max(PER_CORE - t * TILE_P, 0), TILE_P)
        mask[:v, t] = 1.0

    in_maps = []
    for cc in range(N_CORES):
        sl = data[cc * PER_CORE:(cc + 1) * PER_CORE]
        xt = np.zeros((N_FEATURES, PADDED), np.float32)
        xt[:, :PER_CORE] = sl.T
        xt = xt.reshape(N_FEATURES, N_TILES // 2, 2, TILE_P)
        xpc = np.empty((128, PADDED // 2), np.float32)
        xpc[0:64] = xt[:, :, 0, :].reshape(N_FEATURES, -1)
        xpc[64:128] = xt[:, :, 1, :].reshape(N_FEATURES, -1)
        in_maps.append({"xp": xpc.astype(ml_dtypes.bfloat16), "bm": bm,
                        "lconst": lconst, "mask": mask})
    return in_maps


def _run(data, weights, means, covariances, trace=False):
    from concourse.bass_utils import run_bass_kernel_spmd

    data = np.asarray(data, np.float32)
    bm, lconst_row, m0 = _precompute(np.asarray(weights), np.asarray(means),
                                     np.asarray(covariances))
    if "nc" not in _CACHE:
        _CACHE["nc"] = _build_nc()
    nc = _CACHE["nc"]

    in_maps = _make_inputs(data, bm, lconst_row)
    res = run_bass_kernel_spmd(nc, in_maps, list(range(N_CORES)), trace=trace)
    total = 0.0
    for cc in range(N_CORES):
        total += float(res.results[cc]["out"][0, 0]) + PER_CORE * m0
    return np.float32(total), res


def kernel(data, weights, means, covariances):
    return _run(data, weights, means, covariances)[0]
